# revision 22
# baseline (speedup 1.0000x reference)
"""nn_ContrastiveLoss Trainium2 kernel (8 NeuronCores, data-parallel over batch).

Contract: kernel(embeddings=[64,1024,128] f32, labels=[64,1024] int64) -> f32 scalar.

Sharding: batch dim B=64 split as 8 samples per core. Host packs each sample's
rows by label (positives first, then negatives, each zero-padded to a 128-row
multiple) AND permutes rows so the device-side DMA is fully contiguous per
partition (device tile [p, t] = packed row t*128+p lives at host row p*tt+t).

Device pipeline, software-pipelined 3 deep (issue order interleaves samples so
per-engine FIFOs don't serialize the chain):
  phase A(b):   DMA e_nat [128, tt, 128] f32 (5KB contiguous per partition)
  phase B(b-1): ACT Square -> esq bf16; DVE reduce -> nsq; ACT sqrt(+eps^2);
                DVE recip; GPSIMD tensor_mul e_nrm = e_nat * rinv (bf16);
                PE transpose-mode (bf16 PSUM, 1 bank per half);
                DVE 2x copies -> et_p/et_n bf16 SBUF
  phase C(b-2): PE sim matmuls into grouped PSUM tiles [128, 2, 640];
                hinge fused with accum_out: ACT relu(sim-0.15) on groups
                {0,1} and {4}, DVE max(sim,0.15)-sum on group {2,3}
                (constant offset removed host-side)
  tail: ones^T @ slots matmul = partition reduction; DMA [1, bpc*tp] raw slot
  sums. Host: subtract DVE offsets, per-sample division by max(nneg,1),
  validity, and the final count division (exact host arithmetic; counts come
  from labels).
"""

import sys

if "/opt/trn_rl_repo" not in sys.path:
    sys.path.insert(0, "/opt/trn_rl_repo")

from contextlib import ExitStack

import numpy as np

import concourse.bass as bass
import concourse.bacc as bacc
import concourse.mybir as mybir
import concourse.tile as tile
from concourse import bass_utils

F32 = mybir.dt.float32
BF16 = mybir.dt.bfloat16
AF = mybir.ActivationFunctionType
ALU = mybir.AluOpType

P = 128      # SBUF partitions
D = 128      # embedding dim
N = 1024     # rows per sample
B = 64       # full batch
NCORES = 8
BPC = B // NCORES
THRESH = 0.5 - 0.35   # margin threshold 0.15
EPS = 1e-6


def _kernel_body(ctx, tc, emb_ap, out_ap, bpc, padp, padn, padn_eff):
    nc = tc.nc
    tp, tn = padp // P, padn // P
    tt = tp + tn

    const_pool = ctx.enter_context(tc.tile_pool(name="const", bufs=1))
    epool = ctx.enter_context(tc.tile_pool(name="epool", bufs=4))
    etpool = ctx.enter_context(tc.tile_pool(name="etpool", bufs=3))
    small = ctx.enter_context(tc.tile_pool(name="small", bufs=3))
    acc_pool = ctx.enter_context(tc.tile_pool(name="acc", bufs=1))
    tr_psum = ctx.enter_context(tc.tile_pool(name="trps", bufs=2, space="PSUM"))
    sim_psum = ctx.enter_context(tc.tile_pool(name="simps", bufs=3, space="PSUM"))

    neg_thr = const_pool.tile([P, 1], F32)
    nc.gpsimd.memset(neg_thr[:], -THRESH)
    eps2 = const_pool.tile([P, 1], F32)
    nc.gpsimd.memset(eps2[:], EPS * EPS)
    ones_col = const_pool.tile([P, 1], F32)
    nc.gpsimd.memset(ones_col[:], 1.0)
    # bf16 identity for PE transpose mode
    ident = const_pool.tile([P, D], BF16)
    nc.gpsimd.affine_select(
        ident[:], ones_col[:].broadcast_to([P, D]),
        pattern=[[-1, D]], compare_op=ALU.is_equal, fill=0.0,
        base=0, channel_multiplier=1,
    )

    # Dummy activations to pull both ACT table loads into the initial DMA wait.
    warm = const_pool.tile([P, 1], F32)
    nc.scalar.activation(warm[:], eps2[:], AF.Square)
    nc.scalar.activation(warm[:], eps2[:], AF.Sqrt, bias=eps2[:])

    # flat sim layout: tp x padn_eff fp32 linearized into [P, 2, 512] PSUM
    # tiles (2 banks each); hinge runs per tile — slot attribution is
    # irrelevant since S_b sums everything. Engine per tile alternates.
    flat_total = tp * padn_eff
    TILE_W = 1024
    n_simtiles = -(-flat_total // TILE_W)
    tile_widths = [min(TILE_W, flat_total - k * TILE_W)
                   for k in range(n_simtiles)]
    # DVE takes every third tile (ACT is cheaper per element from PSUM)
    tile_eng = ["DVE" if k % 3 == 1 else "ACT" for k in range(n_simtiles)]

    slots_all = acc_pool.tile([P, bpc, n_simtiles], F32)

    e_nats, nsqs, rinvs, e_nrms = {}, {}, {}, {}
    ets = {}

    def phase_dma(b):
        e_nat = epool.tile([P, tt, D], F32, tag="e_nat", name=f"e_nat{b}")
        nc.sync.dma_start(e_nat[:], emb_ap[b])
        e_nats[b] = e_nat

    esqs = {}

    def phase_square(b):
        e_nat = e_nats[b]
        esq = epool.tile([P, tt, D], BF16, tag="esq", name=f"esq{b}")
        nc.scalar.activation(esq[:], e_nat[:], AF.Square)
        esqs[b] = esq

    def phase_reduce(b):
        esq = esqs.pop(b)
        nsq = small.tile([P, tt], F32, tag="nsq", name=f"nsq{b}")
        nc.vector.tensor_reduce(nsq[:], esq[:], axis=mybir.AxisListType.X,
                                op=ALU.add)
        nsqs[b] = nsq

    def phase_scale(b):
        e_nat = e_nats.pop(b)
        nsq = nsqs.pop(b)
        # r = sqrt(nsq + eps^2) folds in the max(r, eps) clamp (pad rows)
        r_ = small.tile([P, tt], F32, tag="r_", name=f"r{b}")
        nc.scalar.activation(r_[:], nsq[:], AF.Sqrt, bias=eps2[:])
        rinv = small.tile([P, tt], F32, tag="rinv", name=f"rinv{b}")
        nc.vector.reciprocal(rinv[:], r_[:])

        # normalized bf16 rows in one gpsimd op (frees ACT/DVE)
        e_nrm = epool.tile([P, tt, D], BF16, tag="e_nrm", name=f"e_nrm{b}")
        nc.gpsimd.tensor_mul(e_nrm[:], e_nat[:],
                             rinv[:].unsqueeze(2).broadcast_to([P, tt, D]))
        e_nrms[b] = e_nrm

    def phase_tr(b):
        e_nrm = e_nrms.pop(b)
        # PE transpose mode -> bf16 PSUM (1 bank per half)
        ps_p = tr_psum.tile([P, padp], BF16, tag="trps", name=f"psp{b}")
        ps_n = tr_psum.tile([P, padn], BF16, tag="trps", name=f"psn{b}")
        for t in range(tp):
            nc.tensor.transpose(ps_p[:, bass.ts(t, P)], e_nrm[:, t, :],
                                ident[:])
        for t in range(tn):
            nc.tensor.transpose(ps_n[:, bass.ts(t, P)], e_nrm[:, tp + t, :],
                                ident[:])
        et_p = etpool.tile([P, padp], BF16, tag="et_p", name=f"etp{b}")
        nc.vector.tensor_copy(et_p[:], ps_p[:])
        et_n = etpool.tile([P, padn_eff], BF16, tag="et_n", name=f"etn{b}")
        nc.vector.tensor_copy(et_n[:], ps_n[:, 0:padn_eff])
        ets[b] = (et_p, et_n)

    def phase_sim(b):
        et_p, et_n = ets.pop(b)
        tiles = [sim_psum.tile([P, 2, 512], F32, tag="simps",
                               name=f"sim{b}_{k}")
                 for k in range(n_simtiles)]
        flat = [t[:].rearrange("p a w -> p (a w)") for t in tiles]
        # walk the flat space; split MMs at 512-fp32 (bank/tile) edges
        for mt in range(tp):
            f0 = mt * padn_eff
            j0 = 0
            while j0 < padn_eff:
                f = f0 + j0
                jw = min(512 - (f % 512), padn_eff - j0)
                k, off = f // TILE_W, f % TILE_W
                nc.tensor.matmul(flat[k][:, off:off + jw],
                                 lhsT=et_p[:, bass.ts(mt, P)],
                                 rhs=et_n[:, j0:j0 + jw],
                                 start=True, stop=True)
                j0 += jw
        for k in range(n_simtiles):
            view = flat[k][:, 0:tile_widths[k]]
            slot = slots_all[:, b, k:k + 1]
            if tile_eng[k] == "ACT":
                nc.scalar.activation(view, view, AF.Relu,
                                     bias=neg_thr[:], accum_out=slot)
            else:
                nc.vector.tensor_scalar(view, view, THRESH, None,
                                        ALU.max, ALU.add, accum_out=slot)

    # 6-deep software pipeline. Issue order within a step is oldest stage
    # first so each strict-FIFO engine queue leads with ready work (avoids
    # head-of-line blocking on same-step cross-engine dependencies).
    stages = [phase_dma, phase_square, phase_reduce, phase_scale, phase_tr,
              phase_sim]
    nst = len(stages)
    for s in range(bpc + nst - 1):
        if 0 <= s - 0 < bpc:
            phase_dma(s)
        for k in range(nst - 1, 0, -1):
            b = s - k
            if 0 <= b < bpc:
                stages[k](b)

    # partition-reduce all slots with one tiny fp32 matmul: ones^T @ slots
    nsl = bpc * n_simtiles
    red_ps = sim_psum.tile([1, nsl], F32, tag="simps")
    nc.tensor.matmul(red_ps[:], lhsT=ones_col[:],
                     rhs=slots_all[:].rearrange("p b t -> p (b t)"),
                     start=True, stop=True)
    out_sb = small.tile([1, nsl], F32, tag="out_sb")
    nc.scalar.copy(out_sb[:], red_ps[:])
    nc.sync.dma_start(out_ap[:], out_sb[:])


_NC_CACHE = {}


def _build(padp, padn, padn_eff):
    key = (BPC, NCORES, padp, padn, padn_eff)
    if key in _NC_CACHE:
        return _NC_CACHE[key]
    tp = padp // P
    nc = bacc.Bacc("TRN2", target_bir_lowering=False, debug=False,
                   num_devices=NCORES)
    tt = (padp + padn) // P
    emb = nc.dram_tensor("emb", [BPC, P, tt, D], F32, kind="ExternalInput")
    n_simtiles = -(-(tp * padn_eff) // 1024)
    out = nc.dram_tensor("out", [1, BPC * n_simtiles], F32,
                         kind="ExternalOutput")
    with tile.TileContext(nc) as tc:
        with ExitStack() as ctx:
            _kernel_body(ctx, tc, emb.ap(), out.ap(), BPC, padp, padn,
                         padn_eff)
    nc.compile()
    _NC_CACHE[key] = nc
    return nc


def _pack(emb, labels):
    """Per-sample label packing: pos rows, zero pad, neg rows, zero pad.

    Rows are additionally permuted so that the device-side DMA of tile
    [p, t] (= packed row t*128+p) reads contiguously: host row p*tt + t.
    """
    npos = (labels == 1).sum(axis=1)
    nneg = (labels == 0).sum(axis=1)
    padp = max(P, int(-(-npos.max() // P)) * P)
    padn = max(P, int(-(-nneg.max() // P)) * P)
    tt = (padp + padn) // P
    packed = np.zeros((B, padp + padn, D), np.float32)
    for b in range(B):
        pos_idx = np.nonzero(labels[b] == 1)[0]
        neg_idx = np.nonzero(labels[b] == 0)[0]
        packed[b, :len(pos_idx)] = emb[b, pos_idx]
        packed[b, padp:padp + len(neg_idx)] = emb[b, neg_idx]
    # [B, tt*P, D] -> [B, tt, P, D] -> [B, P, tt, D]: row t*128+p -> [p, t]
    perm = np.ascontiguousarray(
        packed.reshape(B, tt, P, D).transpose(0, 2, 1, 3))
    return perm, padp, padn, npos, nneg


def _dve_width(tp, padn_eff):
    # mirrors the flat sim tile layout/engine assignment in _kernel_body
    flat_total = tp * padn_eff
    n_simtiles = -(-flat_total // 1024)
    w = 0
    for k in range(n_simtiles):
        if k % 3 == 1:
            w += min(1024, flat_total - k * 1024)
    return n_simtiles, w


def kernel(embeddings: np.ndarray, labels: np.ndarray,
           _want_results=False, _trace=False) -> np.ndarray:
    emb = np.ascontiguousarray(embeddings, dtype=np.float32)
    lab = np.asarray(labels)
    assert emb.shape == (B, N, D) and lab.shape == (B, N)

    perm, padp, padn, npos, nneg = _pack(emb, lab)
    tp = padp // P
    # only the first padn_eff neg columns are real; the rest are always pad
    padn_eff = min(padn, int(-(-int(nneg.max()) // 32)) * 32)
    nc = _build(padp, padn, padn_eff)
    in_maps = [{"emb": perm[c * BPC:(c + 1) * BPC]} for c in range(NCORES)]
    res = bass_utils.run_bass_kernel_spmd(nc, in_maps,
                                          core_ids=list(range(NCORES)),
                                          trace=_trace)

    # host-side: remove DVE max-trick offsets, per-sample division, all-reduce
    n_simtiles, dve_w = _dve_width(tp, padn_eff)
    dve_off = float(P) * float(dve_w) * THRESH
    loss_sum = 0.0
    for c in range(NCORES):
        slots = np.asarray(res.results[c]["out"],
                           np.float64).reshape(BPC, n_simtiles)
        s_raw = slots.sum(axis=1) - dve_off
        for i in range(BPC):
            b = c * BPC + i
            if npos[b] > 0 and nneg[b] > 0:
                loss_sum += s_raw[i] / max(float(nneg[b]), 1.0)
    valid = (npos > 0) & (nneg > 0)
    count = float((npos * valid).sum())
    ans = np.float32(loss_sum / max(count, 1.0))
    if _want_results:
        return ans, res
    return ans


# revision 25
# speedup vs baseline: 1.0784x; 1.0784x over previous
"""nn_ContrastiveLoss Trainium2 kernel (8 NeuronCores, data-parallel over batch).

Contract: kernel(embeddings=[64,1024,128] f32, labels=[64,1024] int64) -> f32 scalar.

Sharding: batch dim B=64 split as 8 samples per core. Host packs each sample's
rows by label (positives first, then negatives, each zero-padded to a 128-row
multiple) AND permutes rows so the device-side DMA is fully contiguous per
partition (device tile [p, t] = packed row t*128+p lives at host row p*tt+t).

Device pipeline, software-pipelined 3 deep (issue order interleaves samples so
per-engine FIFOs don't serialize the chain):
  phase A(b):   DMA e_nat [128, tt, 128] f32 (5KB contiguous per partition)
  phase B(b-1): ACT Square -> esq bf16; DVE reduce -> nsq; ACT sqrt(+eps^2);
                DVE recip; GPSIMD tensor_mul e_nrm = e_nat * rinv (bf16);
                PE transpose-mode (bf16 PSUM, 1 bank per half);
                DVE 2x copies -> et_p/et_n bf16 SBUF
  phase C(b-2): PE sim matmuls into grouped PSUM tiles [128, 2, 640];
                hinge fused with accum_out: ACT relu(sim-0.15) on groups
                {0,1} and {4}, DVE max(sim,0.15)-sum on group {2,3}
                (constant offset removed host-side)
  tail: ones^T @ slots matmul = partition reduction; DMA [1, bpc*tp] raw slot
  sums. Host: subtract DVE offsets, per-sample division by max(nneg,1),
  validity, and the final count division (exact host arithmetic; counts come
  from labels).
"""

import sys

if "/opt/trn_rl_repo" not in sys.path:
    sys.path.insert(0, "/opt/trn_rl_repo")

from contextlib import ExitStack

import numpy as np

import concourse.bass as bass
import concourse.bacc as bacc
import concourse.mybir as mybir
import concourse.tile as tile
from concourse import bass_utils

F32 = mybir.dt.float32
BF16 = mybir.dt.bfloat16
AF = mybir.ActivationFunctionType
ALU = mybir.AluOpType

P = 128      # SBUF partitions
D = 128      # embedding dim
N = 1024     # rows per sample
B = 64       # full batch
NCORES = 8
BPC = B // NCORES
THRESH = 0.5 - 0.35   # margin threshold 0.15
EPS = 1e-6


def _kernel_body(ctx, tc, emb_ap, out_ap, bpc, padp, padn, padn_eff):
    nc = tc.nc
    tp, tn = padp // P, padn // P
    tt = tp + tn

    const_pool = ctx.enter_context(tc.tile_pool(name="const", bufs=1))
    epool = ctx.enter_context(tc.tile_pool(name="epool", bufs=4))
    etpool = ctx.enter_context(tc.tile_pool(name="etpool", bufs=3))
    small = ctx.enter_context(tc.tile_pool(name="small", bufs=3))
    acc_pool = ctx.enter_context(tc.tile_pool(name="acc", bufs=1))
    tr_psum = ctx.enter_context(tc.tile_pool(name="trps", bufs=2, space="PSUM"))
    sim_psum = ctx.enter_context(tc.tile_pool(name="simps", bufs=3, space="PSUM"))

    neg_thr = const_pool.tile([P, 1], F32)
    nc.gpsimd.memset(neg_thr[:], -THRESH)
    eps2 = const_pool.tile([P, 1], F32)
    nc.gpsimd.memset(eps2[:], EPS * EPS)
    ones_col = const_pool.tile([P, 1], F32)
    nc.gpsimd.memset(ones_col[:], 1.0)
    # bf16 identity for PE transpose mode
    ident = const_pool.tile([P, D], BF16)
    nc.gpsimd.affine_select(
        ident[:], ones_col[:].broadcast_to([P, D]),
        pattern=[[-1, D]], compare_op=ALU.is_equal, fill=0.0,
        base=0, channel_multiplier=1,
    )

    # Dummy activations to pull both ACT table loads into the initial DMA wait.
    warm = const_pool.tile([P, 1], F32)
    nc.scalar.activation(warm[:], eps2[:], AF.Square)
    nc.scalar.activation(warm[:], eps2[:], AF.Sqrt, bias=eps2[:])

    # flat sim layout: tp x padn_eff fp32 linearized into [P, 2, 512] PSUM
    # tiles (2 banks each); hinge runs per tile — slot attribution is
    # irrelevant since S_b sums everything. Engine per tile alternates.
    flat_total = tp * padn_eff
    TILE_W = 1024
    n_simtiles = -(-flat_total // TILE_W)
    tile_widths = [min(TILE_W, flat_total - k * TILE_W)
                   for k in range(n_simtiles)]
    # DVE takes every third tile (ACT is cheaper per element from PSUM)
    tile_eng = ["DVE" if k % 3 == 1 else "ACT" for k in range(n_simtiles)]

    slots_all = acc_pool.tile([P, bpc, n_simtiles], F32)

    e_nats, nsqs, rinvs, e_nrms = {}, {}, {}, {}
    ets = {}

    def phase_dma(b):
        e_nat = epool.tile([P, tt, D], F32, tag="e_nat", name=f"e_nat{b}")
        nc.sync.dma_start(e_nat[:, 0:tp, :], emb_ap[b, :, 0:tp, :])
        nc.sync.dma_start(e_nat[:, tp:tt, :], emb_ap[b, :, tp:tt, :])
        e_nats[b] = e_nat

    esqs = {}

    def phase_square(b):
        e_nat = e_nats[b]
        esq = epool.tile([P, tt, D], BF16, tag="esq", name=f"esq{b}")
        nc.scalar.activation(esq[:, 0:tp, :], e_nat[:, 0:tp, :], AF.Square)
        nc.scalar.activation(esq[:, tp:tt, :], e_nat[:, tp:tt, :], AF.Square)
        esqs[b] = esq

    def phase_reduce(b):
        esq = esqs.pop(b)
        nsq = small.tile([P, tt], F32, tag="nsq", name=f"nsq{b}")
        nc.vector.tensor_reduce(nsq[:, 0:tp], esq[:, 0:tp, :],
                                axis=mybir.AxisListType.X, op=ALU.add)
        nc.vector.tensor_reduce(nsq[:, tp:tt], esq[:, tp:tt, :],
                                axis=mybir.AxisListType.X, op=ALU.add)
        nsqs[b] = nsq

    def phase_scale(b):
        e_nat = e_nats.pop(b)
        nsq = nsqs.pop(b)
        # r = sqrt(nsq + eps^2) folds in the max(r, eps) clamp (pad rows)
        r_ = small.tile([P, tt], F32, tag="r_", name=f"r{b}")
        nc.scalar.activation(r_[:], nsq[:], AF.Sqrt, bias=eps2[:])
        rinv = small.tile([P, tt], F32, tag="rinv", name=f"rinv{b}")
        nc.vector.reciprocal(rinv[:], r_[:])

        # normalized bf16 rows on gpsimd, split so transposes start at half
        e_nrm = epool.tile([P, tt, D], BF16, tag="e_nrm", name=f"e_nrm{b}")
        rbc = rinv[:].unsqueeze(2).broadcast_to([P, tt, D])
        nc.gpsimd.tensor_mul(e_nrm[:, 0:tp, :], e_nat[:, 0:tp, :],
                             rbc[:, 0:tp, :])
        nc.gpsimd.tensor_mul(e_nrm[:, tp:tt, :], e_nat[:, tp:tt, :],
                             rbc[:, tp:tt, :])
        e_nrms[b] = e_nrm

    def phase_tr(b):
        e_nrm = e_nrms.pop(b)
        # PE transpose mode -> bf16 PSUM (1 bank per half)
        ps_p = tr_psum.tile([P, padp], BF16, tag="trps", name=f"psp{b}")
        ps_n = tr_psum.tile([P, padn], BF16, tag="trps", name=f"psn{b}")
        for t in range(tp):
            nc.tensor.transpose(ps_p[:, bass.ts(t, P)], e_nrm[:, t, :],
                                ident[:])
        for t in range(tn):
            nc.tensor.transpose(ps_n[:, bass.ts(t, P)], e_nrm[:, tp + t, :],
                                ident[:])
        et_p = etpool.tile([P, padp], BF16, tag="et_p", name=f"etp{b}")
        nc.vector.tensor_copy(et_p[:], ps_p[:])
        et_n = etpool.tile([P, padn_eff], BF16, tag="et_n", name=f"etn{b}")
        nc.vector.tensor_copy(et_n[:], ps_n[:, 0:padn_eff])
        ets[b] = (et_p, et_n)

    def phase_sim(b):
        et_p, et_n = ets.pop(b)
        tiles = [sim_psum.tile([P, 2, 512], F32, tag="simps",
                               name=f"sim{b}_{k}")
                 for k in range(n_simtiles)]
        flat = [t[:].rearrange("p a w -> p (a w)") for t in tiles]
        # walk the flat space; split MMs at 512-fp32 (bank/tile) edges
        for mt in range(tp):
            f0 = mt * padn_eff
            j0 = 0
            while j0 < padn_eff:
                f = f0 + j0
                jw = min(512 - (f % 512), padn_eff - j0)
                k, off = f // TILE_W, f % TILE_W
                nc.tensor.matmul(flat[k][:, off:off + jw],
                                 lhsT=et_p[:, bass.ts(mt, P)],
                                 rhs=et_n[:, j0:j0 + jw],
                                 start=True, stop=True)
                j0 += jw
        for k in range(n_simtiles):
            view = flat[k][:, 0:tile_widths[k]]
            slot = slots_all[:, b, k:k + 1]
            if tile_eng[k] == "ACT":
                nc.scalar.activation(view, view, AF.Relu,
                                     bias=neg_thr[:], accum_out=slot)
            else:
                nc.vector.tensor_scalar(view, view, THRESH, None,
                                        ALU.max, ALU.add, accum_out=slot)

    # 6-deep software pipeline. Issue order within a step is oldest stage
    # first so each strict-FIFO engine queue leads with ready work (avoids
    # head-of-line blocking on same-step cross-engine dependencies).
    stages = [phase_dma, phase_square, phase_reduce, phase_scale, phase_tr,
              phase_sim]
    nst = len(stages)
    for s in range(bpc + nst - 1):
        if 0 <= s - 0 < bpc:
            phase_dma(s)
        for k in range(nst - 1, 0, -1):
            b = s - k
            if 0 <= b < bpc:
                stages[k](b)

    # partition-reduce all slots with one tiny fp32 matmul: ones^T @ slots
    nsl = bpc * n_simtiles
    red_ps = sim_psum.tile([1, nsl], F32, tag="simps")
    nc.tensor.matmul(red_ps[:], lhsT=ones_col[:],
                     rhs=slots_all[:].rearrange("p b t -> p (b t)"),
                     start=True, stop=True)
    out_sb = small.tile([1, nsl], F32, tag="out_sb")
    nc.scalar.copy(out_sb[:], red_ps[:])
    nc.sync.dma_start(out_ap[:], out_sb[:])


_NC_CACHE = {}


def _build(padp, padn, padn_eff):
    key = (BPC, NCORES, padp, padn, padn_eff)
    if key in _NC_CACHE:
        return _NC_CACHE[key]
    tp = padp // P
    nc = bacc.Bacc("TRN2", target_bir_lowering=False, debug=False,
                   num_devices=NCORES)
    tt = (padp + padn) // P
    emb = nc.dram_tensor("emb", [BPC, P, tt, D], F32, kind="ExternalInput")
    n_simtiles = -(-(tp * padn_eff) // 1024)
    out = nc.dram_tensor("out", [1, BPC * n_simtiles], F32,
                         kind="ExternalOutput")
    with tile.TileContext(nc) as tc:
        with ExitStack() as ctx:
            _kernel_body(ctx, tc, emb.ap(), out.ap(), BPC, padp, padn,
                         padn_eff)
    nc.compile()
    _NC_CACHE[key] = nc
    return nc


def _pack(emb, labels):
    """Per-sample label packing: pos rows, zero pad, neg rows, zero pad.

    Rows are additionally permuted so that the device-side DMA of tile
    [p, t] (= packed row t*128+p) reads contiguously: host row p*tt + t.
    """
    npos = (labels == 1).sum(axis=1)
    nneg = (labels == 0).sum(axis=1)
    padp = max(P, int(-(-npos.max() // P)) * P)
    padn = max(P, int(-(-nneg.max() // P)) * P)
    tt = (padp + padn) // P
    packed = np.zeros((B, padp + padn, D), np.float32)
    for b in range(B):
        pos_idx = np.nonzero(labels[b] == 1)[0]
        neg_idx = np.nonzero(labels[b] == 0)[0]
        packed[b, :len(pos_idx)] = emb[b, pos_idx]
        packed[b, padp:padp + len(neg_idx)] = emb[b, neg_idx]
    # [B, tt*P, D] -> [B, tt, P, D] -> [B, P, tt, D]: row t*128+p -> [p, t]
    perm = np.ascontiguousarray(
        packed.reshape(B, tt, P, D).transpose(0, 2, 1, 3))
    return perm, padp, padn, npos, nneg


def _dve_width(tp, padn_eff):
    # mirrors the flat sim tile layout/engine assignment in _kernel_body
    flat_total = tp * padn_eff
    n_simtiles = -(-flat_total // 1024)
    w = 0
    for k in range(n_simtiles):
        if k % 3 == 1:
            w += min(1024, flat_total - k * 1024)
    return n_simtiles, w


def kernel(embeddings: np.ndarray, labels: np.ndarray,
           _want_results=False, _trace=False) -> np.ndarray:
    emb = np.ascontiguousarray(embeddings, dtype=np.float32)
    lab = np.asarray(labels)
    assert emb.shape == (B, N, D) and lab.shape == (B, N)

    perm, padp, padn, npos, nneg = _pack(emb, lab)
    tp = padp // P
    # only the first padn_eff neg columns are real; the rest are always pad
    padn_eff = min(padn, int(-(-int(nneg.max()) // 32)) * 32)
    nc = _build(padp, padn, padn_eff)
    in_maps = [{"emb": perm[c * BPC:(c + 1) * BPC]} for c in range(NCORES)]
    res = bass_utils.run_bass_kernel_spmd(nc, in_maps,
                                          core_ids=list(range(NCORES)),
                                          trace=_trace)

    # host-side: remove DVE max-trick offsets, per-sample division, all-reduce
    n_simtiles, dve_w = _dve_width(tp, padn_eff)
    dve_off = float(P) * float(dve_w) * THRESH
    loss_sum = 0.0
    for c in range(NCORES):
        slots = np.asarray(res.results[c]["out"],
                           np.float64).reshape(BPC, n_simtiles)
        s_raw = slots.sum(axis=1) - dve_off
        for i in range(BPC):
            b = c * BPC + i
            if npos[b] > 0 and nneg[b] > 0:
                loss_sum += s_raw[i] / max(float(nneg[b]), 1.0)
    valid = (npos > 0) & (nneg > 0)
    count = float((npos * valid).sum())
    ans = np.float32(loss_sum / max(count, 1.0))
    if _want_results:
        return ans, res
    return ans
